# revision 1
# baseline (speedup 1.0000x reference)
"""Trainium2 Bass kernel for nn_AttentionBlock (sparse attention with gaussian bias).

Reference computation (per batch b):
    qp = q @ Wq + bq; kp = k @ Wk + bk; vp = v @ Wv + bv          (d_model=512 -> dk=dv=64)
    attn = qp @ kp^T / 8 + g_bias / (2 tau^2); attn[mask] = -inf
    p = softmax(attn, axis=-1)
    out = (p @ vp) @ Wfc + bfc

Sharding: 8 cores = (batch b in 0..3) x (query-half h in 0..1).
Each core computes a [1024, 2048] attention slab. K/V work is split within each
core pair: each core transposes+projects its half of K/V, then kpT / vp are
AllGathered over the pair (small projected tensors instead of raw K/V).

Per-core dataflow (Sq=1024 local, Sk=2048):
  Phase A: PE-transpose q and half of k/v, project:
      qpT[64,1024] = Wq^T qT * (2 tau^2/8) + bq',  kpT_half[64,1024] = Wk^T kT + bk,
      vp_half[1024,64] = v Wv + bv;  AllGather kpT, vp across the pair.
  Phase B per sq-tile [128 rows]:
      psum = qpT^T @ kpT  (+ I_r @ gm accumulate, gm = g_bias - 1e30*mask, f32r)
      e = exp(psum / (2 tau^2)) with row-sum accumulator (ACT, f32r out)
      eT via PE transposes; unnormalized oT[64,sq] = sum_k vp[k,:]^T e[:,k]
      out = (oT^T @ Wfc) * (1/rowsum) + bfc
"""
import numpy as np

B, S, D, DKV = 4, 2048, 512, 64
SQL = S // 2          # query rows per core
SKL = S // 2          # k/v rows loaded per core (pair-sharded)
N_CORES = 8
NT_K = S // 128       # 16 k/v tiles (full)
NG_Q = SQL // 512     # 2 groups of 4 q-tiles
NG_KL = SKL // 512    # 2 groups of local k/v rows

PAIR_KV = True        # split K/V across core pairs + AllGather projections


def _build():
    import concourse.bass as bass
    import concourse.mybir as mybir
    import concourse.tile as tile
    from concourse import bacc

    f32, bf16, u8 = mybir.dt.float32, mybir.dt.bfloat16, mybir.dt.uint8
    f16 = mybir.dt.float16
    f32r = mybir.dt.float32r
    AF = mybir.ActivationFunctionType
    OP = mybir.AluOpType

    nc = bacc.Bacc(num_devices=N_CORES)
    skl = SKL if PAIR_KV else S
    q_ext = nc.declare_dram_parameter("q", [SQL, D], f32, isOutput=False)
    k_ext = nc.declare_dram_parameter("k", [skl, D], f32, isOutput=False)
    v_ext = nc.declare_dram_parameter("v", [skl, D], f32, isOutput=False)
    gb_ext = nc.declare_dram_parameter("gb", [SQL, S], f32, isOutput=False)
    m_ext = nc.declare_dram_parameter("mask", [SQL, S], u8, isOutput=False)
    wq_ext = nc.declare_dram_parameter("Wq", [D, DKV], f32, isOutput=False)
    wk_ext = nc.declare_dram_parameter("Wk", [D, DKV], f32, isOutput=False)
    wv_ext = nc.declare_dram_parameter("Wv", [D, DKV], f32, isOutput=False)
    wfc_ext = nc.declare_dram_parameter("Wfc", [DKV, D], f32, isOutput=False)
    bq_ext = nc.declare_dram_parameter("bq", [DKV, 1], f32, isOutput=False)
    bk_ext = nc.declare_dram_parameter("bk", [DKV, 1], f32, isOutput=False)
    bv_ext = nc.declare_dram_parameter("bvb", [128, DKV], f32, isOutput=False)
    bfc_ext = nc.declare_dram_parameter("bfcb", [128, D], f32, isOutput=False)
    # host-derived scalars: qscale = 2*tau^2/8 (per dk partition), escale = 1/(2 tau^2)
    qs_ext = nc.declare_dram_parameter("qscale", [DKV, 1], f32, isOutput=False)
    es_ext = nc.declare_dram_parameter("escale", [128, 1], f32, isOutput=False)
    out_ext = nc.declare_dram_parameter("out", [SQL, D], f32, isOutput=True)

    # collective bounce buffers (internal DRAM; outs in Shared space)
    if PAIR_KV:
        kp_ag_in = nc.dram_tensor("kp_ag_in", [DKV, SKL], f32r)
        kp_ag_out = nc.dram_tensor("kp_ag_out", [2, DKV, SKL], f32r)
        vp_ag_in = nc.dram_tensor("vp_ag_in", [128, NT_K // 2, DKV], mybir.dt.float16)
        vp_ag_out = nc.dram_tensor("vp_ag_out", [2, 128, NT_K // 2, DKV], mybir.dt.float16)
        pair_groups = [[2 * b, 2 * b + 1] for b in range(4)]

    with tile.TileContext(nc) as tc:
        from contextlib import ExitStack
        with ExitStack() as ctx:
            wpool = ctx.enter_context(tc.tile_pool(name="weights", bufs=1))
            proj_pool = ctx.enter_context(tc.tile_pool(name="proj", bufs=1))

            # ---- small weights / constants ----
            wq_t = wpool.tile([128, 4, DKV], f32, tag="wq")
            wk_t = wpool.tile([128, 4, DKV], f32, tag="wk")
            wv_t = wpool.tile([128, 4, DKV], f32, tag="wv")
            nc.sync.dma_start(wq_t[:], wq_ext.rearrange("(c p) n -> p c n", p=128))
            nc.sync.dma_start(wk_t[:], wk_ext.rearrange("(c p) n -> p c n", p=128))
            nc.sync.dma_start(wv_t[:], wv_ext.rearrange("(c p) n -> p c n", p=128))
            wfc_t = wpool.tile([DKV, D], f32, tag="wfc")
            nc.sync.dma_start(wfc_t[:], wfc_ext[:])
            bq_t = wpool.tile([DKV, 1], f32, tag="bq")
            bk_t = wpool.tile([DKV, 1], f32, tag="bk")
            bv_t = wpool.tile([128, DKV], f32, tag="bv")
            bfc_t = wpool.tile([128, D], f32, tag="bfc")
            qs_t = wpool.tile([DKV, 1], f32, tag="qs")
            es_t = wpool.tile([128, 1], f32, tag="es")
            nc.sync.dma_start(bq_t[:], bq_ext[:])
            nc.sync.dma_start(bk_t[:], bk_ext[:])
            nc.sync.dma_start(bv_t[:], bv_ext[:])
            nc.sync.dma_start(bfc_t[:], bfc_ext[:])
            nc.sync.dma_start(qs_t[:], qs_ext[:])
            nc.sync.dma_start(es_t[:], es_ext[:])

            # rounded weights for matmuls
            wq_r = wpool.tile([128, 4, DKV], f32r, tag="wq_r")
            wk_r = wpool.tile([128, 4, DKV], f32r, tag="wk_r")
            wfc_r = wpool.tile([DKV, D], f32r, tag="wfc_r")
            nc.vector.tensor_copy(wq_r[:], wq_t[:])
            nc.vector.tensor_copy(wk_r[:], wk_t[:])
            nc.vector.tensor_copy(wfc_r[:], wfc_t[:])

            # identities: f32 for qkv transposes, bf16 for gm add, f16 for eT
            ident = wpool.tile([128, 128], f32, tag="ident")
            ident_bf = wpool.tile([128, 128], bf16, tag="ident_bf")
            ident_h = wpool.tile([128, 128], f16, tag="ident_h")
            from concourse.masks import make_identity
            make_identity(nc, ident[:])
            nc.vector.tensor_copy(ident_bf[:], ident[:])
            nc.vector.tensor_copy(ident_h[:], ident[:])
            eb_t = wpool.tile([128, 1], f32, tag="eb")
            nc.gpsimd.memset(eb_t[:], -3.0)

            # ---- persistent projected tensors (local half computed here, remote
            # half arrives via pair AllGather; sk axis is host-permuted so the
            # local half always occupies columns 0:1024) ----
            kpT_loc = proj_pool.tile([DKV, SKL], f32r, tag="kpT_loc")
            kpT_rem = proj_pool.tile([DKV, SKL], f32r, tag="kpT_rem")
            qpT = proj_pool.tile([DKV, SQL], f32r, tag="qpT")       # [64, 1024]
            vp_loc = proj_pool.tile([128, NT_K // 2, DKV], f16, tag="vp_loc")
            vp_rem = proj_pool.tile([128, NT_K // 2, DKV], f16, tag="vp_rem")

            with tc.tile_pool(name="pa_sbuf", bufs=4) as pa_pool, \
                 tc.tile_pool(name="pa_psumT", bufs=3, space="PSUM") as pa_psT, \
                 tc.tile_pool(name="pa_psumP", bufs=2, space="PSUM") as pa_psP:

                def load_transpose_group(x_ext, g, dt_out, tag, copy_eng, dma_eng):
                    """Load 512 rows of x (one DMA), transpose on PE.
                    Returns xT_sb [128, 4, 512]: chunk j holds xT[d_chunk_j, 512 rows]."""
                    x_t = pa_pool.tile([128, 4, D], f32, tag="x_in")
                    dma_eng(x_t[:],
                            x_ext[512 * g:512 * (g + 1), :]
                            .rearrange("(t p) d -> p t d", p=128))
                    xT_sb = pa_pool.tile([128, 4, 512], dt_out, tag=tag)
                    for t in range(4):
                        ps = pa_psT.tile([128, 4, 128], f32, tag="psT")
                        for j in range(4):
                            nc.tensor.transpose(
                                ps[:, j, :], x_t[:, t, 128 * j:128 * (j + 1)], ident[:])
                        copy_eng(xT_sb[:, :, 128 * t:128 * (t + 1)], ps[:])
                    return xT_sb

                # K local half: kpT_loc [64, SKL]
                ng_k = NG_KL
                for g in range(ng_k):
                    kT = load_transpose_group(k_ext, g, f32r, "xTr",
                                              nc.scalar.copy, nc.sync.dma_start)
                    pp = pa_psP.tile([DKV, 512], f32, tag="psP")
                    for j in range(4):
                        nc.tensor.matmul(pp[:], wk_r[:, j, :], kT[:, j, :],
                                         start=(j == 0), stop=(j == 3))
                    nc.vector.tensor_scalar(
                        out=kpT_loc[:, 512 * g:512 * (g + 1)], in0=pp[:],
                        scalar1=bk_t[:], scalar2=None, op0=OP.add)

                # exchange: send local half, fetch partner half (dynamic row)
                remote_row = 1 - (nc.sync.partition_id() % 2)
                nc.sync.dma_start(kp_ag_in[:], kpT_loc[:])
                nc.gpsimd.collective_compute(
                    "AllGather", OP.bypass, replica_groups=pair_groups,
                    ins=[kp_ag_in.ap()], outs=[kp_ag_out.ap()])
                nc.sync.dma_start(kpT_rem[:], kp_ag_out[bass.ds(remote_row, 1)].squeeze(0))

                # Q: qpT[64, 1024] scaled by 2 tau^2 / 8
                for g in range(NG_Q):
                    qT = load_transpose_group(q_ext, g, f32r, "xTr",
                                              nc.vector.tensor_copy, nc.sync.dma_start)
                    pp = pa_psP.tile([DKV, 512], f32, tag="psP")
                    for j in range(4):
                        nc.tensor.matmul(pp[:], wq_r[:, j, :], qT[:, j, :],
                                         start=(j == 0), stop=(j == 3))
                    nc.vector.tensor_scalar(
                        out=qpT[:, 512 * g:512 * (g + 1)], in0=pp[:],
                        scalar1=bq_t[:], scalar2=qs_t[:], op0=OP.add, op1=OP.mult)

                # V local half: vp natural [skl, dv], f32r, +bv
                for g in range(ng_k):
                    vT = load_transpose_group(v_ext, g, f32, "xTv",
                                              nc.scalar.copy, nc.sync.dma_start)
                    for t in range(4):
                        pv = pa_psP.tile([128, DKV], f32, tag="psV")
                        for j in range(4):
                            nc.tensor.matmul(
                                pv[:], vT[:, j, 128 * t:128 * (t + 1)], wv_t[:, j, :],
                                start=(j == 0), stop=(j == 3))
                        nc.vector.tensor_tensor(
                            out=vp_loc[:, 4 * g + t, :], in0=pv[:], in1=bv_t[:],
                            op=OP.add)

                nc.sync.dma_start(vp_ag_in[:], vp_loc[:])
                nc.gpsimd.collective_compute(
                    "AllGather", OP.bypass, replica_groups=pair_groups,
                    ins=[vp_ag_in.ap()], outs=[vp_ag_out.ap()])
                nc.sync.dma_start(vp_rem[:], vp_ag_out[bass.ds(remote_row, 1)].squeeze(0))

            # ---- phase B ----
            with tc.tile_pool(name="pb_sbuf", bufs=2) as pb_pool, \
                 tc.tile_pool(name="pb_ebuf", bufs=5) as pb_epool, \
                 tc.tile_pool(name="pb_eT", bufs=1) as pb_eTpool, \
                 tc.tile_pool(name="pb_acc", bufs=8) as pb_accpool, \
                 tc.tile_pool(name="pb_ps_s", bufs=2, space="PSUM") as pb_ps_s, \
                 tc.tile_pool(name="pb_ps_eT", bufs=2, space="PSUM") as pb_ps_eT, \
                 tc.tile_pool(name="pb_ps_pv", bufs=1, space="PSUM") as pb_ps_pv, \
                 tc.tile_pool(name="pb_ps_fc", bufs=1, space="PSUM") as pb_ps_fc:

                recips = []
                for g in range(NG_Q):
                    e_tiles = []
                    for t in range(4):
                        i = 4 * g + t
                        sq0 = 128 * i
                        gb_t = pb_pool.tile([128, S], f32, tag="gb")
                        m_bf = pb_pool.tile([128, S], bf16, tag="m")
                        nc.scalar.dma_start(gb_t[:], gb_ext[sq0:sq0 + 128, :])
                        nc.gpsimd.dma_start(m_bf[:], m_ext[sq0:sq0 + 128, :])
                        gm = pb_pool.tile([128, S], bf16, tag="gm")
                        nc.vector.scalar_tensor_tensor(
                            out=gm[:], in0=m_bf[:], scalar=-1e30, in1=gb_t[:],
                            op0=OP.mult, op1=OP.add)

                        e_bf = pb_epool.tile([128, S], f16, tag="e")
                        accs = []
                        for h, kp_half in ((0, kpT_loc), (1, kpT_rem)):
                            hs = slice(1024 * h, 1024 * (h + 1))
                            ps_s = pb_ps_s.tile([128, 1024], f32, tag="score")
                            for c in range(2):
                                sl = slice(1024 * h + 512 * c, 1024 * h + 512 * (c + 1))
                                ksl = slice(512 * c, 512 * (c + 1))
                                psl = slice(512 * c, 512 * (c + 1))
                                nc.tensor.matmul(ps_s[:, psl],
                                                 qpT[:, sq0:sq0 + 128], kp_half[:, ksl],
                                                 start=True, stop=False)
                                nc.tensor.matmul(ps_s[:, psl], ident_bf[:], gm[:, sl],
                                                 start=False, stop=True)
                            acc = pb_accpool.tile([128, 1], f32, tag=f"acc{h}")
                            nc.scalar.activation(e_bf[:, hs], ps_s[:], AF.Exp,
                                                 bias=eb_t[:], scale=es_t[:],
                                                 accum_out=acc[:])
                            accs.append(acc)
                        acc_t = pb_accpool.tile([128, 1], f32, tag="accsum")
                        nc.vector.tensor_tensor(out=acc_t[:], in0=accs[0][:],
                                                in1=accs[1][:], op=OP.add)
                        r_t = pb_accpool.tile([128, 1], f32, tag="recip")
                        nc.vector.reciprocal(r_t[:], acc_t[:])
                        recips.append(r_t)
                        e_tiles.append(e_bf)

                    # eT for the group: eT_sb[:, j, :] = e[512 rows, sk chunk j].T
                    eT_sb = pb_eTpool.tile([128, NT_K, 512], f16, tag="eT")
                    for j in range(NT_K):
                        ps_eT = pb_ps_eT.tile([128, 512], f16, tag="pseT")
                        for t in range(4):
                            nc.tensor.transpose(
                                ps_eT[:, 128 * t:128 * (t + 1)],
                                e_tiles[t][:, 128 * j:128 * (j + 1)], ident_h[:])
                        nc.vector.tensor_copy(eT_sb[:, j, :], ps_eT[:])

                    # PV: oT[64, 512] = sum_j vp_j^T @ eT_j
                    ps_pv = pb_ps_pv.tile([DKV, 512], f32, tag="pspv")
                    for j in range(NT_K):
                        vp_j = vp_loc[:, j, :] if j < NT_K // 2 else vp_rem[:, j - NT_K // 2, :]
                        nc.tensor.matmul(ps_pv[:], vp_j, eT_sb[:, j, :],
                                         start=(j == 0), stop=(j == NT_K - 1))
                    aoT = pb_pool.tile([DKV, 512], f32r, tag="aoT")
                    nc.scalar.copy(aoT[:], ps_pv[:])

                    # FC + normalize + bias + store
                    for t in range(4):
                        i = 4 * g + t
                        ps_fc = pb_ps_fc.tile([128, D], f32, tag="psfc")
                        nc.tensor.matmul(ps_fc[:], aoT[:, 128 * t:128 * (t + 1)],
                                         wfc_r[:], start=True, stop=True)
                        o_sb = pb_pool.tile([128, D], f32, tag="osb")
                        nc.vector.scalar_tensor_tensor(
                            out=o_sb[:], in0=ps_fc[:], scalar=recips[i][:],
                            in1=bfc_t[:], op0=OP.mult, op1=OP.add)
                        nc.sync.dma_start(out_ext[128 * i:128 * (i + 1), :], o_sb[:])

    nc.finalize()
    return nc


_cache = {}


def kernel(**inputs):
    from concourse.bass_utils import run_bass_kernel_spmd

    q = np.asarray(inputs["q"], np.float32)
    k = np.asarray(inputs["k"], np.float32)
    v = np.asarray(inputs["v"], np.float32)
    gb = np.asarray(inputs["g_bias"], np.float32)
    mask = np.asarray(inputs["mask"]).astype(np.uint8)
    tau = float(np.asarray(inputs["tau"]))

    if "nc" not in _cache:
        _cache["nc"] = _build()
    nc = _cache["nc"]

    in_maps = build_in_maps(inputs, q, k, v, gb, mask, tau)
    res = run_bass_kernel_spmd(nc, in_maps, list(range(N_CORES)))
    out = np.empty((B, S, D), np.float32)
    for c in range(N_CORES):
        b, h = divmod(c, 2)
        out[b, h * SQL:(h + 1) * SQL] = res.results[c]["out"]
    return out


def _perm_cols(x, h):
    """Put the core's local sk-half (columns h*1024:(h+1)*1024) first."""
    if h == 0:
        return np.ascontiguousarray(x)
    return np.ascontiguousarray(np.concatenate([x[:, SKL:], x[:, :SKL]], axis=1))


def build_in_maps(inputs, q, k, v, gb, mask, tau):
    qscale = np.full((DKV, 1), (2.0 * tau * tau) / 8.0, np.float32)
    escale = np.full((128, 1), 1.0 / (2.0 * tau * tau), np.float32)
    shared = {
        "Wq": np.asarray(inputs["Wq"], np.float32),
        "Wk": np.asarray(inputs["Wk"], np.float32),
        "Wv": np.asarray(inputs["Wv"], np.float32),
        "Wfc": np.asarray(inputs["Wfc"], np.float32),
        "bq": np.asarray(inputs["bq"], np.float32).reshape(DKV, 1).copy(),
        "bk": np.asarray(inputs["bk"], np.float32).reshape(DKV, 1).copy(),
        "bvb": np.broadcast_to(np.asarray(inputs["bv"], np.float32), (128, DKV)).copy(),
        "bfcb": np.broadcast_to(np.asarray(inputs["bfc"], np.float32), (128, D)).copy(),
        "qscale": qscale, "escale": escale,
    }
    in_maps = []
    for c in range(N_CORES):
        b, h = divmod(c, 2)
        sl = slice(h * SQL, (h + 1) * SQL)
        ksl = sl if PAIR_KV else slice(None)
        in_maps.append({
            "q": np.ascontiguousarray(q[b, sl]),
            "k": np.ascontiguousarray(k[b, ksl]),
            "v": np.ascontiguousarray(v[b, ksl]),
            "gb": _perm_cols(gb[b, sl], h),
            "mask": _perm_cols(mask[b, sl], h),
            **shared,
        })
    return in_maps



# revision 13
# speedup vs baseline: 1.1122x; 1.1122x over previous
"""Trainium2 Bass kernel for nn_AttentionBlock (sparse attention w/ gaussian bias).

Reference computation (per batch b):
    qp = q @ Wq + bq; kp = k @ Wk + bk; vp = v @ Wv + bv          (d_model=512 -> dk=dv=64)
    attn = qp @ kp^T / 8 + g_bias / (2 tau^2); attn[mask] = -inf
    p = softmax(attn, axis=-1)
    out = (p @ vp) @ Wfc + bfc

Sharding: 8 cores = (batch b in 0..3) x (query-half h in 0..1), fully
independent (K/V replicated per core; no collectives).

Layout strategy: everything is computed TRANSPOSED so no big PE transposes
are needed anywhere:
  - q/k/v are host-cast to bf16 and loaded pre-transposed via DMA-transpose
    (xT[d, rows]); projections contract d directly.
  - scores are built transposed, one sk-tile [128, 1024] at a time:
        sT[128 sk, 1024 sq] = kpT_chunk.T @ qpT  (+ I @ gmT accumulate)
    where gmT = g_bias^T (bf16, host-transposed) - 1e30*mask^T.
  - e = exp(sT/(2 tau^2) - 3) in f16; PV accumulates oT[65, 1024] over all
    16 sk-tiles in one PSUM tile; row 64 (ones-column of V) = softmax
    denominators for free.
  - FC consumes oT directly as lhsT; per-row 1/denominator applied on the
    FC output; denominators extracted with 8 tiny PE transposes.

Scheduling: DMA ring FIFOs and the DVE queue are manually ordered so the
critical chain (kT -> kpT, qT -> qpT, gb0/mask0 -> gm0) clears by ~10us and
the 16 score tiles then pace the kernel; V is projected per-tile inside the
main loop since vT lands after scores begin.
"""
import numpy as np

B, S, D, DKV = 4, 2048, 512, 64
SQL = S // 2          # query rows per core
NT_SK = S // 128      # 16 sk tiles
NG_GB = 4             # gb/mask DMA groups (4 sk-tiles each)
N_CORES = 8


def _build():
    import concourse.bass as bass
    import concourse.mybir as mybir
    import concourse.tile as tile
    from concourse import bacc
    from concourse.masks import make_identity
    from contextlib import ExitStack

    f32, bf16, u8 = mybir.dt.float32, mybir.dt.bfloat16, mybir.dt.uint8
    f16 = mybir.dt.float16
    f32r = mybir.dt.float32r
    AF = mybir.ActivationFunctionType
    OP = mybir.AluOpType

    nc = bacc.Bacc(num_devices=N_CORES)
    q_ext = nc.declare_dram_parameter("q", [SQL, D], bf16, isOutput=False)
    k_ext = nc.declare_dram_parameter("k", [S, D], bf16, isOutput=False)
    v_ext = nc.declare_dram_parameter("v", [S, D], bf16, isOutput=False)
    gbt_ext = nc.declare_dram_parameter("gbt", [S, SQL], bf16, isOutput=False)
    mt_ext = nc.declare_dram_parameter("maskt", [S, SQL], u8, isOutput=False)
    # w3 = [Wq | Wk | Wv] stacked on the output dim, bf16
    w3_ext = nc.declare_dram_parameter("w3", [D, 3 * DKV], bf16, isOutput=False)
    wfc_ext = nc.declare_dram_parameter("Wfc", [DKV, D], f32, isOutput=False)
    # consts = [bq | bk | qscale | escale | bvb | bfcb] packed, f32
    NC_CONST = 4 + DKV + D
    consts_ext = nc.declare_dram_parameter("consts", [128, NC_CONST], f32,
                                           isOutput=False)
    out_ext = nc.declare_dram_parameter("out", [SQL, D], f32, isOutput=True)

    with tile.TileContext(nc) as tc:
        with ExitStack() as ctx:
            wpool = ctx.enter_context(tc.tile_pool(name="weights", bufs=1))
            ppool = ctx.enter_context(tc.tile_pool(name="proj", bufs=1))

            # ---- packed weights / constants (gpsimd ring) ----
            w3_t = wpool.tile([128, 4, 3 * DKV], bf16, tag="w3")
            nc.gpsimd.dma_start(w3_t[:], w3_ext.rearrange("(c p) n -> p c n", p=128))
            wq_t = w3_t[:, :, 0:DKV]
            wk_t = w3_t[:, :, DKV:2 * DKV]
            wv_t = w3_t[:, :, 2 * DKV:3 * DKV]
            consts_t = wpool.tile([128, NC_CONST], f32, tag="consts")
            nc.gpsimd.dma_start(consts_t[:], consts_ext[:])
            bq_t = consts_t[0:DKV, 0:1]
            bk_t = consts_t[0:DKV, 1:2]
            qs_t = consts_t[0:DKV, 2:3]
            es_t = consts_t[:, 3:4]
            bv_t = consts_t[:, 4:4 + DKV]
            bfc_t = consts_t[:, 4 + DKV:NC_CONST]

            ident = wpool.tile([128, 128], f32, tag="ident")
            make_identity(nc, ident[:])
            ident_bf = wpool.tile([128, 128], bf16, tag="ident_bf")
            ident_r = wpool.tile([128, 128], f32r, tag="ident_r")
            nc.vector.tensor_copy(ident_bf[:], ident[:])
            nc.vector.tensor_copy(ident_r[:], ident[:])
            eb_t = wpool.tile([128, 1], f32, tag="eb")
            nc.gpsimd.memset(eb_t[:], -3.0)

            # ---- persistent projected tensors ----
            kpT = ppool.tile([DKV, S], f32r, tag="kpT")        # [64, 2048]
            qpT = ppool.tile([DKV, SQL], f32r, tag="qpT")      # [64, 1024]
            vp_aug = ppool.tile([128, NT_SK, DKV + 1], f16, tag="vp")
            # 66 partitions: fp32r matmul/transpose ISA checks need even sizes
            oT_sb = ppool.tile([DKV + 2, SQL], f32r, tag="oT")
            recip_sb = ppool.tile([128, SQL // 128], f32, tag="recip")
            wfc_t = wpool.tile([DKV, D], f32, tag="wfc")
            wfc_r = wpool.tile([DKV, D], f32r, tag="wfc_r")

            nc.gpsimd.memset(vp_aug[:, :, DKV:DKV + 1], 1.0)

            with tc.tile_pool(name="pa_kv", bufs=1) as pkv_pool, \
                 tc.tile_pool(name="pa_q", bufs=1) as pq_pool, \
                 tc.tile_pool(name="pb_gb", bufs=2) as pgb, \
                 tc.tile_pool(name="pb_m", bufs=2) as pm, \
                 tc.tile_pool(name="pb_gm", bufs=3) as pgm, \
                 tc.tile_pool(name="pb_e", bufs=4) as pe_pool, \
                 tc.tile_pool(name="pb_o", bufs=2) as po_pool:

                # ---- transposed loads: each split across both HWDGE rings ----
                kT = pkv_pool.tile([128, 4, S], bf16, tag="kT")
                for j in range(4):
                    nc.sync.dma_start_transpose(
                        kT[:, j, :], k_ext[:, 128 * j:128 * (j + 1)])
                qT = pq_pool.tile([128, 4, SQL], bf16, tag="qT")
                for j in range(4):
                    nc.sync.dma_start_transpose(
                        qT[:, j, :], q_ext[:, 128 * j:128 * (j + 1)])

                # gb/mask DMAs: group 0 on the (otherwise idle early) gpsimd
                # ring; groups 1-3 on sync behind the transposes.
                gb_tiles, m_tiles = {}, {}

                def issue_group_dma(g, gb_eng):
                    rows = slice(512 * g, 512 * (g + 1))
                    gb_t = pgb.tile([128, 4, SQL], bf16, tag="gb")
                    gb_eng.dma_start(
                        gb_t[:], gbt_ext[rows].rearrange("(t p) s -> p t s", p=128))
                    m_t = pm.tile([128, 4, SQL], bf16, tag="m")
                    nc.gpsimd.dma_start(
                        m_t[:], mt_ext[rows].rearrange("(t p) s -> p t s", p=128))
                    gb_tiles[g], m_tiles[g] = gb_t, m_t

                issue_group_dma(0, nc.gpsimd)
                nc.gpsimd.dma_start(wfc_t[:], wfc_ext[:])
                issue_group_dma(1, nc.gpsimd)

                gm_tiles = {}

                def fuse_half(g, half):
                    """gm for tiles (4g + 2*half) and (4g + 2*half + 1)."""
                    gm_t = pgm.tile([128, 2, SQL], bf16, tag="gm")
                    csl = slice(2 * half, 2 * half + 2)
                    nc.vector.scalar_tensor_tensor(
                        out=gm_t[:], in0=m_tiles[g][:, csl, :], scalar=-1e30,
                        in1=gb_tiles[g][:, csl, :], op0=OP.mult, op1=OP.add)
                    gm_tiles[(g, half)] = gm_t

                vT = pkv_pool.tile([128, 4, S], bf16, tag="vT")
                for j in range(4):
                    nc.sync.dma_start_transpose(
                        vT[:, j, :], v_ext[:, 128 * j:128 * (j + 1)])

                # ---- k/q projections (own PSUM scope) ----
                with tc.tile_pool(name="pa_ps", bufs=2, space="PSUM") as pa_ps:
                    # kpT half 0 first: unblocks score tiles 0..7
                    def kproj(half):
                        hsl = slice(SQL * half, SQL * (half + 1))
                        pp = pa_ps.tile([DKV, SQL], f32, tag="pp")
                        for c in range(2):
                            sl = slice(512 * c, 512 * (c + 1))
                            psl = slice(SQL * half + 512 * c,
                                        SQL * half + 512 * (c + 1))
                            for j in range(4):
                                nc.tensor.matmul(pp[:, sl], wk_t[:, j, :],
                                                 kT[:, j, psl],
                                                 start=(j == 0), stop=(j == 3))
                        nc.vector.tensor_scalar(out=kpT[:, hsl], in0=pp[:],
                                                scalar1=bk_t, scalar2=None,
                                                op0=OP.add)

                    kproj(0)
                    fuse_half(0, 0)

                    pp = pa_ps.tile([DKV, SQL], f32, tag="pp")
                    for c in range(2):
                        sl = slice(512 * c, 512 * (c + 1))
                        for j in range(4):
                            nc.tensor.matmul(pp[:, sl], wq_t[:, j, :], qT[:, j, sl],
                                             start=(j == 0), stop=(j == 3))
                    nc.vector.tensor_scalar(out=qpT[:], in0=pp[:], scalar1=bq_t,
                                            scalar2=qs_t, op0=OP.add, op1=OP.mult)
                    fuse_half(0, 1)
                    kproj(1)

                # ---- phase B: scores, exp, per-tile V projection, PV ----
                with tc.tile_pool(name="ps_s", bufs=2, space="PSUM") as ps_s, \
                     tc.tile_pool(name="ps_pv", bufs=1, space="PSUM") as ps_pv, \
                     tc.tile_pool(name="ps_fc", bufs=2, space="PSUM") as ps_fc:

                    oT_ps = ps_pv.tile([DKV + 1, SQL], f32, tag="oTp")

                    for g in range(NG_GB):
                        for t in range(4):
                            j = 4 * g + t
                            gm_t = gm_tiles[(g, t // 2)]
                            ps = ps_s.tile([128, SQL], f32, tag="s")
                            for c in range(2):
                                sl = slice(512 * c, 512 * (c + 1))
                                nc.tensor.matmul(ps[:, sl],
                                                 kpT[:, 128 * j:128 * (j + 1)],
                                                 qpT[:, sl], start=True, stop=False)
                            for c in range(2):
                                sl = slice(512 * c, 512 * (c + 1))
                                nc.tensor.matmul(ps[:, sl], ident_bf[:],
                                                 gm_t[:, t % 2, sl],
                                                 start=False, stop=True)
                            # V projection for this sk tile (vT lands late;
                            # keeps PE fed without blocking early scores)
                            pv = ps_fc.tile([128, DKV], f32, tag="fc")
                            for vj in range(4):
                                nc.tensor.matmul(pv[:],
                                                 vT[:, vj, 128 * j:128 * (j + 1)],
                                                 wv_t[:, vj, :], start=(vj == 0),
                                                 stop=(vj == 3))
                            nc.vector.tensor_tensor(out=vp_aug[:, j, 0:DKV],
                                                    in0=pv[:], in1=bv_t, op=OP.add)
                            e_t = pe_pool.tile([128, SQL], f16, tag="e")
                            nc.scalar.activation(e_t[:], ps[:], AF.Exp,
                                                 bias=eb_t[:], scale=es_t)
                            for c in range(2):
                                sl = slice(512 * c, 512 * (c + 1))
                                nc.tensor.matmul(oT_ps[:, sl], vp_aug[:, j, :],
                                                 e_t[:, sl], start=(j == 0),
                                                 stop=(j == NT_SK - 1))
                            # stage upcoming groups
                            if t == 0 and g + 2 < NG_GB:
                                issue_group_dma(g + 2, nc.sync)
                            if t == 1 and g + 1 < NG_GB:
                                fuse_half(g + 1, 0)
                            if t == 3 and g + 1 < NG_GB:
                                fuse_half(g + 1, 1)

                    # ---- tail: denominators, FC, store ----
                    nc.vector.tensor_copy(wfc_r[:], wfc_t[:])
                    nc.scalar.copy(oT_sb[0:DKV + 1, :], oT_ps[:])
                    for t in range(SQL // 128):
                        csl = slice(128 * t, 128 * (t + 1))
                        tr = ps_fc.tile([128, DKV + 2], f32r, tag="fc")
                        nc.tensor.transpose(tr[:], oT_sb[:, csl],
                                            ident_r[0:DKV + 2, 0:DKV + 2])
                        nc.vector.reciprocal(recip_sb[:, t:t + 1],
                                             tr[:, DKV:DKV + 1])
                    for t in range(SQL // 128):
                        csl = slice(128 * t, 128 * (t + 1))
                        pf = ps_fc.tile([128, D], f32, tag="fc")
                        nc.tensor.matmul(pf[:], oT_sb[0:DKV, csl], wfc_r[:],
                                         start=True, stop=True)
                        o_sb = po_pool.tile([128, D], f32, tag="o")
                        nc.vector.scalar_tensor_tensor(
                            out=o_sb[:], in0=pf[:], scalar=recip_sb[:, t:t + 1],
                            in1=bfc_t, op0=OP.mult, op1=OP.add)
                        nc.sync.dma_start(out_ext[csl, :], o_sb[:])

    nc.finalize()
    return nc


_cache = {}


def _bf16(x):
    import ml_dtypes
    return np.ascontiguousarray(np.asarray(x, np.float32).astype(ml_dtypes.bfloat16))


def kernel(**inputs):
    from concourse.bass_utils import run_bass_kernel_spmd

    q = np.asarray(inputs["q"], np.float32)
    k = np.asarray(inputs["k"], np.float32)
    v = np.asarray(inputs["v"], np.float32)
    gb = np.asarray(inputs["g_bias"], np.float32)
    mask = np.asarray(inputs["mask"]).astype(np.uint8)
    tau = float(np.asarray(inputs["tau"]))

    if "nc" not in _cache:
        _cache["nc"] = _build()
    nc = _cache["nc"]

    in_maps = build_in_maps(inputs, q, k, v, gb, mask, tau)
    res = run_bass_kernel_spmd(nc, in_maps, list(range(N_CORES)))
    out = np.empty((B, S, D), np.float32)
    for c in range(N_CORES):
        b, h = divmod(c, 2)
        out[b, h * SQL:(h + 1) * SQL] = res.results[c]["out"]
    return out


def build_in_maps(inputs, q, k, v, gb, mask, tau):
    consts = np.zeros((128, 4 + DKV + D), np.float32)
    consts[0:DKV, 0] = np.asarray(inputs["bq"], np.float32)
    consts[0:DKV, 1] = np.asarray(inputs["bk"], np.float32)
    consts[0:DKV, 2] = (2.0 * tau * tau) / 8.0
    consts[:, 3] = 1.0 / (2.0 * tau * tau)
    consts[:, 4:4 + DKV] = np.asarray(inputs["bv"], np.float32)
    consts[:, 4 + DKV:] = np.asarray(inputs["bfc"], np.float32)
    w3 = np.concatenate([np.asarray(inputs["Wq"], np.float32),
                         np.asarray(inputs["Wk"], np.float32),
                         np.asarray(inputs["Wv"], np.float32)], axis=1)
    shared = {
        "w3": _bf16(w3),
        "Wfc": np.asarray(inputs["Wfc"], np.float32),
        "consts": consts,
    }
    kb = [_bf16(k[b]) for b in range(B)]
    vb = [_bf16(v[b]) for b in range(B)]
    in_maps = []
    for c in range(N_CORES):
        b, h = divmod(c, 2)
        sl = slice(h * SQL, (h + 1) * SQL)
        in_maps.append({
            "q": _bf16(q[b, sl]),
            "k": kb[b],
            "v": vb[b],
            "gbt": _bf16(gb[b, sl].T),
            "maskt": np.ascontiguousarray(mask[b, sl].T),
            **shared,
        })
    return in_maps
